# revision 1
# baseline (speedup 1.0000x reference)
"""Multi-head attention (B=2, S=2048, D=1024, H=16) on 8 Trainium2 cores.

Sharding: core = 4*b + g  (b = batch 0..1, g = head-group 0..3, 4 heads each).
Each core computes, for its batch b and head-group g (256 of the 1024 dims):
  QT/KT = (x @ W^T)^T  in [d, s] layout   (d on partitions)
  V     = x @ W^T      in [s, d] layout   (s on partitions)
  ST    = scores^T     in [k, q] layout   (k on partitions)  -> exp on ACT
  U     = V^T @ P^T    in [d, q] layout + per-head denominators Z via ones-matmul
  UN    = U / Z        (PE-broadcast reciprocal, DVE multiply)
  Ypart = UN^T @ WoT   in [q, e] layout   (partial over this group's 256 dims)
Host sums the 4 per-group partials per batch and adds b_o.

All matmuls run in bfloat16 (1 cycle/row, FWL weight loads).
"""

import os
from contextlib import ExitStack

import ml_dtypes
import numpy as np

import concourse.bass as bass
import concourse.tile as tile
from concourse import bacc, mybir
from concourse.tile import add_dep_helper

B, S, D = 2, 2048, 1024
H, DH = 16, 64
NCORES = 8
NG = 4                  # head-group shards
DG = D // NG            # 256 dims per head-group (4 heads)
P = 128
QC = 512                # q-chunk width
NQC = S // QC           # 4
NKT = S // P            # 16 k-tiles of 128
CD = D // P             # 8 contraction tiles for the projections
F32 = mybir.dt.float32
BF16 = mybir.dt.bfloat16
AF = mybir.ActivationFunctionType
SCALE = 1.0 / float(np.sqrt(D))





def _body(ctx: ExitStack, tc: "tile.TileContext", io: dict):
    nc = tc.nc
    # bf16 operands feed the PE at full rate (1 cycle/row + fast weight load);
    # accumulation stays fp32 in PSUM.
    ctx.enter_context(nc.allow_low_precision(reason="bf16 matmul pipeline"))
    sb = ctx.enter_context(tc.tile_pool(name="sb", bufs=1))

    # --- constants (memset can't target f32r; DMA from a ones input) -------
    ones_col = sb.tile([1, P], BF16, tag="ones_col", bufs=1, name="ones_col")
    nc.sync.dma_start(ones_col[:], io["ones"][None, :])

    # --- biases ------------------------------------------------------------
    bq = sb.tile([P, 2], F32, tag="bq", bufs=1, name="bq")
    nc.sync.dma_start(bq[:], io["bq"].rearrange("(t p) -> p t", p=P))
    bk = sb.tile([P, 2], F32, tag="bk", bufs=1, name="bk")
    nc.sync.dma_start(bk[:], io["bk"].rearrange("(t p) -> p t", p=P))
    bv_row = sb.tile([1, DG], BF16, tag="bv", bufs=1, name="bv_row")
    nc.sync.dma_start(bv_row[:], io["bv"][None, :])

    # --- output projection weights -----------------------------------------
    woT = []
    for pr in range(2):
        t = sb.tile([P, D], BF16, tag="wo", bufs=2, name=f"woT{pr}")
        nc.sync.dma_start(t[:], io["wo"][pr * P : (pr + 1) * P, :])
        woT.append(t)

    # --- phase 1: projections (own PSUM pool, 8 banks) ---------------------
    # Order Q -> V -> K: attention's PSUM pool allocation waits for this
    # pool's release, so the last projection should be the one attention
    # needs first (K chunk 0 for the first scores).
    QT, KT = {}, {}
    V = {}

    def qk_proj(ps1, nm, xkey, wkey, bias, outmap):
        w = sb.tile([P, CD, DG], BF16, tag="w", bufs=2, name=f"w{nm}")
        nc.sync.dma_start(w[:], io[wkey].rearrange("(c p) d -> p c d", p=P))
        psg = {}
        for d in range(2):
            for sc in range(NQC):
                psg[d, sc] = ps1.tile(
                    [P, QC], F32, tag="proj", bufs=8, name=f"ps_{nm}{d}{sc}"
                )
        for c in range(CD):
            xt = sb.tile([P, S], BF16, tag="x", bufs=8, name=f"x{nm}{c}")
            nc.sync.dma_start(xt[:], io[xkey][c * P : (c + 1) * P, :])
            for d in range(2):
                for sc in range(NQC):
                    nc.tensor.matmul(
                        psg[d, sc][:],
                        (w[:, c, d * P : (d + 1) * P]),
                        (xt[:, sc * QC : (sc + 1) * QC]),
                        start=(c == 0),
                        stop=(c == CD - 1),
                    )
        for d in range(2):
            for sc in range(NQC):
                t = sb.tile([P, QC], BF16, tag=f"{nm}t", bufs=8, name=f"{nm}T{d}{sc}")
                nc.vector.tensor_scalar_add(t[:], psg[d, sc][:], bias[:, d : d + 1])
                outmap[d, sc] = t

    with tc.tile_pool(name="ps_proj", bufs=1, space="PSUM") as ps1:
        qk_proj(ps1, "q", "xq", "wq", bq, QT)

        # V projection: V [2048, 256] as 16 tiles of [128, 256]; bias b_v is
        # folded in by seeding each PSUM accumulation with ones_col^T @ bv_row.
        wv = sb.tile([P, CD, DG], BF16, tag="w", bufs=2, name="wv")
        nc.sync.dma_start(wv[:], io["wv"].rearrange("(c p) d -> p c d", p=P))
        psv = {
            sp: ps1.tile([P, 2, DG], F32, tag="proj", bufs=8, name=f"psv{sp}")
            for sp in range(8)
        }
        for sp in range(8):
            seed = None
            for j in range(2):
                mm = nc.tensor.matmul(
                    psv[sp][:, j, :],
                    (ones_col[:, 0:P]),
                    (bv_row[:]),
                    start=(j == 0),
                    stop=False,
                )
                # start=True must execute before any other matmul in the bank;
                # disjoint-slice writes carry no natural dep, so add one.
                if j == 0:
                    seed = mm
                else:
                    add_dep_helper(mm.ins, seed.ins, reason="psum group order")
        last_j0 = {}
        for c in range(CD):
            xt = sb.tile([P, S], BF16, tag="x", bufs=8, name=f"xv{c}")
            nc.sync.dma_start(xt[:], io["xv"][c * P : (c + 1) * P, :])
            for sp in range(8):
                for j in range(2):
                    st_i = sp * 2 + j
                    mm = nc.tensor.matmul(
                        psv[sp][:, j, :],
                        (xt[:, st_i * P : (st_i + 1) * P]),
                        (wv[:, c, :]),
                        start=False,
                        stop=(c == CD - 1 and j == 1),
                    )
                    if j == 0:
                        last_j0[sp] = mm
                    elif c == CD - 1:
                        # stop=True closes the whole bank's group; it must run
                        # after the other slice's last matmul.
                        add_dep_helper(mm.ins, last_j0[sp].ins, reason="psv stop order")
        # V_aug tiles [128, 4, 65]: per head 64 V columns + a ones column that
        # accumulates the softmax denominator into row 64 of U_h.
        ones4 = sb.tile([P, 4], BF16, tag="ones4", bufs=1, name="ones4")
        nc.sync.dma_start(ones4[:], io["ones4"][:])
        for sp in range(8):
            for j in range(2):
                vt = sb.tile([P, 4, DH + 1], BF16, tag="v", bufs=16, name=f"V{sp}_{j}")
                nc.vector.tensor_copy(
                    vt[:, :, 0:DH],
                    psv[sp][:, j, :].rearrange("p (g d) -> p g d", g=4),
                )
                nc.vector.tensor_copy(vt[:, :, DH : DH + 1], ones4[:, :, None])
                V[sp * 2 + j] = vt

        qk_proj(ps1, "k", "xk", "wk", bk, KT)

    # --- attention, per q-chunk --------------------------------------------
    # Per-head PSUM accumulators U_h [65, 512]: rows 0..63 are sum_k P*V, row
    # 64 is the softmax denominator (from V_aug's ones column). All matmul
    # outputs start at partition 0 (col-offset tile_position fails walrus
    # codegen in this toolchain).
    ps2 = ctx.enter_context(tc.tile_pool(name="ps_attn", bufs=1, space="PSUM"))
    UN = {}
    YSB = {}
    pending = []

    def emit_outproj_unit():
        if not pending:
            return
        qcp, qi, ec = pending.pop(0)
        qt = qcp * 4 + qi
        if ec == 0:
            YSB[qt] = sb.tile([P, D], F32, tag="y", bufs=6, name=f"Y{qt}")
        ysb = YSB[qt]
        yps = ps2.tile([P, QC], F32, tag="st", bufs=3, name=f"yp{qt}_{ec}")
        for pr in range(2):
            nc.tensor.matmul(
                yps[:],
                (UN[qcp, pr][:, qi * P : (qi + 1) * P]),
                (woT[pr][:, ec * QC : (ec + 1) * QC]),
                start=(pr == 0),
                stop=(pr == 1),
            )
        nc.vector.tensor_copy(ysb[:, ec * QC : (ec + 1) * QC], yps[:])
        if ec == 1:
            nc.sync.dma_start(io["y"][qt * P : (qt + 1) * P, :], ysb[:])

    for qc in range(NQC):
        # Head-pairs are processed in serial k-sweeps: only 2 U accumulator
        # banks live at a time, which frees PSUM for 3 ST slots (6 banks) so
        # the PE can run further ahead of the exp pipeline and stay warm.
        # Pair-0 normalization overlaps pair-1's k-sweep.
        for pair in range(2):
            heads = (2 * pair, 2 * pair + 1)
            U = {
                h: ps2.tile([P, QC], F32, tag="u", bufs=2, name=f"U{qc}_{h}")
                for h in heads
            }
            for kg in range(NKT // 2):
                for h in heads:
                    pr, lo = h // 2, (h % 2) * 64
                    st2 = ps2.tile(
                        [P, 2, QC], F32, tag="st", bufs=3, name=f"st{qc}_{kg}_{h}"
                    )
                    for kk in range(2):
                        k_tile = kg * 2 + kk
                        sc, off = divmod(k_tile, 4)
                        nc.tensor.matmul(
                            st2[:, kk, :],
                            (KT[pr, sc][lo : lo + 64, off * P : (off + 1) * P]),
                            (QT[pr, qc][lo : lo + 64, :]),
                            start=True,
                            stop=True,
                            tile_position=(lo, 0),
                        )
                    pt2 = sb.tile(
                        [P, 2, QC], BF16, tag="pt", bufs=8, name=f"pt{qc}_{kg}_{h}"
                    )
                    nc.scalar.activation(pt2[:], st2[:], AF.Exp, scale=SCALE)
                    for kk in range(2):
                        k_tile = kg * 2 + kk
                        nc.tensor.matmul(
                            U[h][0:65, :],
                            (V[k_tile][:, h, :]),
                            (pt2[:, kk, :]),
                            start=(kg == 0 and kk == 0),
                            stop=(kg == NKT // 2 - 1 and kk == 1),
                        )
                # one out-projection unit of a previous q-chunk every other
                # k-group: independent PE filler while ACT runs exp.
                if kg % 2 == pair:
                    emit_outproj_unit()

            # normalize this pair: UN rows = U_h[0:64] * (1/Z_h); the odd
            # head's rows are DMA-shifted into partitions 64..127.
            UN[qc, pair] = sb.tile(
                [P, QC], BF16, tag="un", bufs=8, name=f"UN{qc}_{pair}"
            )
            z2 = sb.tile([2, QC], F32, tag="z4", bufs=3, name=f"z2_{qc}_{pair}")
            for i, h in enumerate(heads):
                zs = sb.tile([65, QC], F32, tag="zs", bufs=3, name=f"zs{qc}_{h}")
                nc.vector.tensor_copy(zs[64:65, :], U[h][64:65, :])
                nc.sync.dma_start(z2[i : i + 1, :], zs[64:65, :])
            rz2 = sb.tile([2, QC], F32, tag="rz4", bufs=3, name=f"rz2_{qc}_{pair}")
            nc.vector.reciprocal(rz2[:], z2[:])
            for i, h in enumerate(heads):
                off = (h % 2) * 64
                if i == 0:
                    r0 = rz2[0:1, :]
                else:
                    r0t = sb.tile([1, QC], F32, tag="r0", bufs=3, name=f"r0_{qc}_{h}")
                    nc.sync.dma_start(r0t[:], rz2[1:2, :])
                    r0 = r0t[:]
                rb = sb.tile([64, QC], F32, tag="rb", bufs=4, name=f"rb{qc}_{h}")
                nc.gpsimd.partition_broadcast(rb[:], r0, channels=64)
                if off == 0:
                    nc.vector.tensor_mul(UN[qc, pair][0:64, :], U[h][0:64, :], rb[:])
                else:
                    tmp = sb.tile(
                        [64, QC], BF16, tag="untmp", bufs=3, name=f"untmp{qc}_{h}"
                    )
                    nc.vector.tensor_mul(tmp[:], U[h][0:64, :], rb[:])
                    nc.sync.dma_start(UN[qc, pair][64:128, :], tmp[:])

        pending.extend((qc, qi, ec) for qi in range(4) for ec in range(2))

    while pending:
        emit_outproj_unit()


def build_program():
    nc = bacc.Bacc(
        "TRN2", target_bir_lowering=False, debug=False, num_devices=NCORES
    )
    io = {
        "xq": nc.dram_tensor("xq", [D, S], BF16, kind="ExternalInput").ap(),
        "xk": nc.dram_tensor("xk", [D, S], BF16, kind="ExternalInput").ap(),
        "xv": nc.dram_tensor("xv", [D, S], BF16, kind="ExternalInput").ap(),
        "wq": nc.dram_tensor("wq", [D, DG], BF16, kind="ExternalInput").ap(),
        "wk": nc.dram_tensor("wk", [D, DG], BF16, kind="ExternalInput").ap(),
        "wv": nc.dram_tensor("wv", [D, DG], BF16, kind="ExternalInput").ap(),
        "wo": nc.dram_tensor("wo", [DG, D], BF16, kind="ExternalInput").ap(),
        "bq": nc.dram_tensor("bq", [DG], F32, kind="ExternalInput").ap(),
        "bk": nc.dram_tensor("bk", [DG], F32, kind="ExternalInput").ap(),
        "bv": nc.dram_tensor("bv", [DG], BF16, kind="ExternalInput").ap(),
        "ones": nc.dram_tensor("ones", [P], BF16, kind="ExternalInput").ap(),
        "ones4": nc.dram_tensor("ones4", [P, 4], BF16, kind="ExternalInput").ap(),
        "y": nc.dram_tensor("y", [S, D], F32, kind="ExternalOutput").ap(),
    }
    with tile.TileContext(nc) as tc:
        with ExitStack() as ctx:
            _body(ctx, tc, io)
    nc.compile()
    return nc


_CACHE = {}


def _get_program():
    if "nc" not in _CACHE:
        _CACHE["nc"] = build_program()
    return _CACHE["nc"]


def make_in_maps(inputs):
    q = np.asarray(inputs["query"], np.float32)
    k = np.asarray(inputs["key"], np.float32)
    v = np.asarray(inputs["value"], np.float32)
    W_q = np.asarray(inputs["W_q"], np.float32)
    W_k = np.asarray(inputs["W_k"], np.float32)
    W_v = np.asarray(inputs["W_v"], np.float32)
    W_o = np.asarray(inputs["W_o"], np.float32)
    b_q = np.asarray(inputs["b_q"], np.float32)
    b_k = np.asarray(inputs["b_k"], np.float32)
    b_v = np.asarray(inputs["b_v"], np.float32)

    bf = ml_dtypes.bfloat16
    xT = [
        [np.ascontiguousarray(x[b].T).astype(bf) for b in range(B)]
        for x in (q, k, v)
    ]
    in_maps = []
    for core in range(NCORES):
        b, g = divmod(core, NG)
        sl = slice(g * DG, (g + 1) * DG)
        in_maps.append(
            {
                "xq": xT[0][b],
                "xk": xT[1][b],
                "xv": xT[2][b],
                "wq": np.ascontiguousarray(W_q[sl, :].T).astype(bf),
                "wk": np.ascontiguousarray(W_k[sl, :].T).astype(bf),
                "wv": np.ascontiguousarray(W_v[sl, :].T).astype(bf),
                "wo": np.ascontiguousarray(W_o[:, sl].T).astype(bf),
                "bq": np.ascontiguousarray(b_q[sl]),
                "bk": np.ascontiguousarray(b_k[sl]),
                "bv": np.ascontiguousarray(b_v[sl]).astype(bf),
                "ones": np.ones(P, bf),
                "ones4": np.ones((P, 4), bf),
            }
        )
    return in_maps


def kernel(**inputs):
    from concourse.bass_utils import run_bass_kernel_spmd

    nc = _get_program()
    in_maps = make_in_maps(inputs)
    trace = bool(int(os.environ.get("MHA_TRACE", "0")))
    res = run_bass_kernel_spmd(nc, in_maps, list(range(NCORES)), trace=trace)
    _CACHE["last_results"] = res

    b_o = np.asarray(inputs["b_o"], np.float32)
    out = np.zeros((B, S, D), np.float32)
    for core in range(NCORES):
        b = core // NG
        out[b] += res.results[core]["y"]
    out += b_o[None, None, :]
    return out



# revision 5
# speedup vs baseline: 1.0864x; 1.0864x over previous
"""Multi-head attention (B=2, S=2048, D=1024, H=16) on 8 Trainium2 cores.

Sharding: core = 4*b + g  (b = batch 0..1, g = head-group 0..3, 4 heads each).
Heads are processed in pairs; pair p covers the group's d-dims [128p, 128p+128).

Schedule: the scalar engine's exp stream (128 activations of [128,1024],
~172us) is the critical resource.  A short DMA-led preamble projects only
K(pair0), Q(pair0,qc0), V(kt0,1); then 64 pipelined rounds (one per
(pair, qc, k-group)) keep ACT continuously busy:

  round r: [forced proj groups] scores(r) -> exp(r) -> PV(r-1)
           [normalize at sweep boundaries] [filler: proj / out-proj]

All other projections (K pair1, remaining Q, V) and the output projection
run as PE filler inside the rounds' slack so the tensor engine never idles
(and stays at full DVFS pstate).  Sweep order is pair-major so pair1's
weights/projections have 4 sweeps of slack to materialize.

Exactness notes: b_k only shifts each softmax row uniformly -> dropped.
b_v and b_o commute with softmax-average -> folded into the host reduce.
b_q is applied on-device (fused into the Q PSUM->SBUF copy).
"""

import os
from collections import defaultdict, deque
from contextlib import ExitStack

import ml_dtypes
import numpy as np

import concourse.bass as bass
import concourse.tile as tile
from concourse import bacc, mybir

B, S, D = 2, 2048, 1024
H, DH = 16, 64
NCORES = 8
NG = 4                  # head-group shards
DG = D // NG            # 256 dims per head-group (4 heads, 2 pairs)
P = 128
QC = 512                # q-chunk width
NQC = S // QC           # 4
NKT = S // P            # 16 k-tiles of 128
CD = D // P             # 8 contraction tiles for the projections
NR = 64                 # pipeline rounds: 2 pairs x 4 qc x 8 k-groups
F32 = mybir.dt.float32
BF16 = mybir.dt.bfloat16
AF = mybir.ActivationFunctionType
SCALE = 1.0 / float(np.sqrt(D))


def _body(ctx: ExitStack, tc: "tile.TileContext", io: dict):
    nc = tc.nc
    ctx.enter_context(nc.allow_low_precision(reason="bf16 matmul pipeline"))
    sb = ctx.enter_context(tc.tile_pool(name="sb", bufs=1))
    ps = ctx.enter_context(tc.tile_pool(name="ps", bufs=1, space="PSUM"))

    # ---------------- DMA: inputs stream in consumption order --------------
    xk_sb, xq_sb, xv_sb = {}, {}, {}
    w_sb = {}

    def dma_x(dst_map, key, idx):
        t = sb.tile([P, CD, QC], BF16, tag="x", bufs=12, name=f"{key}{idx}")
        # two halves so projection groups can start on the first half
        nc.sync.dma_start(t[:, 0:4, :], io[key][idx, :, 0:4, :])
        nc.sync.dma_start(t[:, 4:8, :], io[key][idx, :, 4:8, :])
        dst_map[idx] = t

    def dma_w(kind, pr):
        t = sb.tile([P, CD, P], BF16, tag="w", bufs=6, name=f"w{kind}{pr}")
        nc.sync.dma_start(t[:], io[f"w{kind}"][pr])
        w_sb[(kind, pr)] = t

    dma_w("k", 0)
    dma_x(xk_sb, "xk", 0)
    dma_w("q", 0)
    bq = sb.tile([P, 2], F32, tag="bq", bufs=1, name="bq")
    nc.sync.dma_start(bq[:], io["bq"])
    ones2 = sb.tile([P, 2], BF16, tag="ones2", bufs=1, name="ones2")
    nc.sync.dma_start(ones2[:], io["ones2"])
    dma_x(xk_sb, "xk", 1)
    dma_x(xq_sb, "xq", 0)
    dma_w("v", 0)
    dma_x(xv_sb, "xv", 0)
    dma_x(xv_sb, "xv", 1)
    dma_x(xk_sb, "xk", 2)
    dma_x(xv_sb, "xv", 2)
    dma_x(xk_sb, "xk", 3)
    dma_x(xv_sb, "xv", 3)
    dma_x(xq_sb, "xq", 1)
    dma_w("k", 1)
    dma_w("q", 1)
    dma_x(xq_sb, "xq", 2)
    dma_w("v", 1)
    dma_x(xq_sb, "xq", 3)
    woT = []
    for pr in range(2):
        t = sb.tile([P, D], BF16, tag="wo", bufs=2, name=f"woT{pr}")
        nc.sync.dma_start(t[:], io["wo"][pr])
        woT.append(t)

    # ---------------- projection / out-proj emitters -----------------------
    KT, QT, V, UN, YSB = {}, {}, {}, {}, {}

    def emit_qk_group(kind, pr, idx):
        w = w_sb[(kind, pr)]
        x = (xk_sb if kind == "k" else xq_sb)[idx]
        pg = ps.tile([P, QC], F32, tag="pj", bufs=1, name=f"pg{kind}{pr}{idx}")
        for c in range(CD):
            nc.tensor.matmul(
                pg[:], w[:, c, :], x[:, c, :], start=(c == 0), stop=(c == CD - 1)
            )
        t = sb.tile([P, QC], BF16, tag=f"{kind}t", bufs=8, name=f"{kind}T{pr}_{idx}")
        if kind == "q":
            nc.vector.tensor_scalar_add(t[:], pg[:], bq[:, pr : pr + 1])
            QT[(pr, idx)] = t
        else:
            nc.vector.tensor_copy(t[:], pg[:])
            KT[(pr, idx)] = t

    def emit_v_group(pair, kt):
        sc, off = divmod(kt, 4)
        x = xv_sb[sc]
        pg = ps.tile([P, P], F32, tag="pj", bufs=1, name=f"pgv{pair}{kt}")
        for c in range(CD):
            nc.tensor.matmul(
                pg[:],
                x[:, c, off * P : (off + 1) * P],
                w_sb[("v", pair)][:, c, :],
                start=(c == 0),
                stop=(c == CD - 1),
            )
        vt = sb.tile([P, 2, DH + 1], BF16, tag="v", bufs=32, name=f"V{pair}_{kt}")
        nc.vector.tensor_copy(vt[:, :, 0:DH], pg[:].rearrange("p (i d) -> p i d", i=2))
        nc.vector.tensor_copy(vt[:, :, DH : DH + 1], ones2[:, :, None])
        V[(pair, kt)] = vt

    pending = deque()

    def emit_outproj_unit(tag="pj"):
        qc, qi, ec = pending.popleft()
        qt = qc * 4 + qi
        if ec == 0:
            YSB[qt] = sb.tile([P, D], BF16, tag="y", bufs=4, name=f"Y{qt}")
        ysb = YSB[qt]
        yp = ps.tile([P, QC], F32, tag=tag, bufs=(1 if tag == "pj" else 3), name=f"yp{qt}_{ec}")
        for pr in range(2):
            nc.tensor.matmul(
                yp[:],
                UN[(qc, pr)][:, qi * P : (qi + 1) * P],
                woT[pr][:, ec * QC : (ec + 1) * QC],
                start=(pr == 0),
                stop=(pr == 1),
            )
        nc.vector.tensor_copy(ysb[:, ec * QC : (ec + 1) * QC], yp[:])
        if ec == 1:
            nc.sync.dma_start(io["y"][qt * P : (qt + 1) * P, :], ysb[:])

    # ---------------- attention round emitters -----------------------------
    PTs, U = {}, {}

    def sweep_of(r):
        s = r // 8
        pair, qc = divmod(s, 4)
        return s, pair, qc, r % 8

    def emit_scores(r):
        _, pair, qc, kg = sweep_of(r)
        for i in (0, 1):
            lo = 64 * i
            st = ps.tile([P, 2, QC], F32, tag="st", bufs=2, name=f"st{r}_{i}")
            for kk in (0, 1):
                kt = kg * 2 + kk
                sc, off = divmod(kt, 4)
                nc.tensor.matmul(
                    st[:, kk, :],
                    KT[(pair, sc)][lo : lo + 64, off * P : (off + 1) * P],
                    QT[(pair, qc)][lo : lo + 64, :],
                    start=True,
                    stop=True,
                    tile_position=(lo, 0),
                )
            pt = sb.tile([P, 2, QC], BF16, tag="pt", bufs=4, name=f"pt{r}_{i}")
            nc.scalar.activation(pt[:], st[:], AF.Exp, scale=SCALE)
            PTs[(r, i)] = pt

    def emit_pv(r):
        s, pair, qc, kg = sweep_of(r)
        if kg == 0:
            U[s] = [
                ps.tile([P, QC], F32, tag="u", bufs=3, name=f"U{s}_{i}") for i in (0, 1)
            ]
        for i in (0, 1):
            pt = PTs.pop((r, i))
            for kk in (0, 1):
                kt = kg * 2 + kk
                nc.tensor.matmul(
                    U[s][i][0:65, :],
                    V[(pair, kt)][:, i, :],
                    pt[:, kk, :],
                    start=(kg == 0 and kk == 0),
                    stop=(kg == 7 and kk == 1),
                )

    def emit_normalize(s):
        pair, qc = divmod(s, 4)
        un = sb.tile([P, QC], BF16, tag="un", bufs=8, name=f"UN{qc}_{pair}")
        for i in (0, 1):
            u = U[s][i]
            zr = sb.tile([65, QC], F32, tag="zr", bufs=4, name=f"zr{s}_{i}")
            nc.vector.tensor_copy(zr[64:65, :], u[64:65, :])
            z = sb.tile([1, QC], F32, tag="z", bufs=4, name=f"z{s}_{i}")
            nc.sync.dma_start(z[:], zr[64:65, :])
            rz = sb.tile([1, QC], F32, tag="rz", bufs=4, name=f"rz{s}_{i}")
            nc.vector.reciprocal(rz[:], z[:])
            rb = sb.tile([64, QC], F32, tag="rb", bufs=4, name=f"rb{s}_{i}")
            nc.gpsimd.partition_broadcast(rb[:], rz[:], channels=64)
            if i == 0:
                nc.vector.tensor_mul(un[0:64, :], u[0:64, :], rb[:])
            else:
                tmp = sb.tile([64, QC], BF16, tag="untmp", bufs=2, name=f"ut{s}")
                nc.vector.tensor_mul(tmp[:], u[0:64, :], rb[:])
                nc.sync.dma_start(un[64:128, :], tmp[:])
        del U[s]
        UN[(qc, pair)] = un

    # ---------------- static schedule --------------------------------------
    # Fillers: (cols, closure); forced at their deadline round if not yet run.
    class Job:
        __slots__ = ("cols", "fn", "done")

        def __init__(self, cols, fn):
            self.cols, self.fn, self.done = cols, fn, False

        def run(self):
            if not self.done:
                self.done = True
                self.fn()

    def qk_job(kind, pr, idx):
        return Job(4096, lambda: emit_qk_group(kind, pr, idx))

    jobs = {}
    for kind, pr, idx in [
        ("q", 0, 1), ("q", 0, 2), ("q", 0, 3),
        ("k", 1, 0), ("k", 1, 1), ("k", 1, 2), ("k", 1, 3),
        ("q", 1, 0), ("q", 1, 1), ("q", 1, 2), ("q", 1, 3),
    ]:
        jobs[(kind, pr, idx)] = qk_job(kind, pr, idx)

    # EDF-ordered general filler queue with earliest-emission gates
    # (rounds before which the needed DMA has not landed yet).
    fq = deque(
        [
            (8, jobs[("q", 0, 1)]),
            (9, jobs[("k", 1, 0)]),
            (10, jobs[("q", 1, 0)]),
            (10, jobs[("q", 0, 2)]),
            (10, jobs[("k", 1, 1)]),
            (11, jobs[("k", 1, 2)]),
            (11, jobs[("k", 1, 3)]),
            (12, jobs[("q", 0, 3)]),
            (13, jobs[("q", 1, 1)]),
            (13, jobs[("q", 1, 2)]),
            (14, jobs[("q", 1, 3)]),
        ]
    )

    # mand_pre: tiles this round's scores read -> must be emitted first.
    jobs[("k", 0, 2)] = qk_job("k", 0, 2)
    jobs[("k", 0, 3)] = qk_job("k", 0, 3)
    mand_pre = defaultdict(list)
    mand_pre[2].append(jobs[("k", 0, 2)])
    mand_pre[4].append(jobs[("k", 0, 3)])
    mand_pre[8].append(jobs[("q", 0, 1)])
    mand_pre[16].append(jobs[("q", 0, 2)])
    mand_pre[24].append(jobs[("q", 0, 3)])
    mand_pre[32].append(jobs[("k", 1, 0)])
    mand_pre[32].append(jobs[("q", 1, 0)])
    mand_pre[34].append(jobs[("k", 1, 1)])
    mand_pre[36].append(jobs[("k", 1, 2)])
    mand_pre[38].append(jobs[("k", 1, 3)])
    mand_pre[40].append(jobs[("q", 1, 1)])
    mand_pre[48].append(jobs[("q", 1, 2)])
    mand_pre[56].append(jobs[("q", 1, 3)])

    # mand_post: V just-in-time (consumed by PV one/two rounds later).
    mand_post = defaultdict(list)
    for r in range(7):  # pair0 kt2..15
        mand_post[r].extend(
            Job(1024, (lambda p, k: (lambda: emit_v_group(p, k)))(0, kt))
            for kt in (2 * r + 2, 2 * r + 3)
        )
    for j, r in enumerate(range(24, 32)):  # pair1 kt0..15
        mand_post[r].extend(
            Job(1024, (lambda p, k: (lambda: emit_v_group(p, k)))(1, kt))
            for kt in (2 * j, 2 * j + 1)
        )

    # ---------------- preamble ---------------------------------------------
    emit_qk_group("k", 0, 0)
    emit_qk_group("k", 0, 1)
    emit_qk_group("q", 0, 0)
    emit_v_group(0, 0)
    emit_v_group(0, 1)

    # ---------------- main pipeline ----------------------------------------
    BUDGET = 2200
    for r in range(NR):
        for job in mand_pre[r]:
            job.run()
        emit_scores(r)
        if r > 0:
            emit_pv(r - 1)
        if r % 8 == 0 and r > 0:
            s = r // 8 - 1
            emit_normalize(s)
            pair, qc = divmod(s, 4)
            if pair == 1:
                pending.extend((qc, qi, ec) for qi in range(4) for ec in range(2))
        for job in mand_post[r]:
            job.run()
        budget = BUDGET
        while budget > 0:
            while fq and fq[0][1].done:
                fq.popleft()
            if fq and fq[0][0] <= r:
                _, job = fq.popleft()
                budget -= job.cols
                job.run()
            elif pending:
                emit_outproj_unit()
                budget -= 1024
            else:
                break

    # ---------------- drain -------------------------------------------------
    emit_pv(NR - 1)
    emit_normalize(7)
    pending.extend((3, qi, ec) for qi in range(4) for ec in range(2))
    for _, job in fq:
        job.run()
    tags = ["pj", "u", "u", "u"]
    i = 0
    while pending:
        emit_outproj_unit(tag=tags[i % 4])
        i += 1


def build_program():
    nc = bacc.Bacc(
        "TRN2", target_bir_lowering=False, debug=False, num_devices=NCORES
    )
    io = {
        "xq": nc.dram_tensor("xq", [NQC, P, CD, QC], BF16, kind="ExternalInput").ap(),
        "xk": nc.dram_tensor("xk", [NQC, P, CD, QC], BF16, kind="ExternalInput").ap(),
        "xv": nc.dram_tensor("xv", [NQC, P, CD, QC], BF16, kind="ExternalInput").ap(),
        "wq": nc.dram_tensor("wq", [2, P, CD, P], BF16, kind="ExternalInput").ap(),
        "wk": nc.dram_tensor("wk", [2, P, CD, P], BF16, kind="ExternalInput").ap(),
        "wv": nc.dram_tensor("wv", [2, P, CD, P], BF16, kind="ExternalInput").ap(),
        "wo": nc.dram_tensor("wo", [2, P, D], BF16, kind="ExternalInput").ap(),
        "bq": nc.dram_tensor("bq", [P, 2], F32, kind="ExternalInput").ap(),
        "ones2": nc.dram_tensor("ones2", [P, 2], BF16, kind="ExternalInput").ap(),
        "y": nc.dram_tensor("y", [S, D], BF16, kind="ExternalOutput").ap(),
    }
    with tile.TileContext(nc) as tc:
        with ExitStack() as ctx:
            _body(ctx, tc, io)
    nc.compile()
    return nc


_CACHE = {}


def _get_program():
    if "nc" not in _CACHE:
        _CACHE["nc"] = build_program()
    return _CACHE["nc"]


def make_in_maps(inputs):
    q = np.asarray(inputs["query"], np.float32)
    k = np.asarray(inputs["key"], np.float32)
    v = np.asarray(inputs["value"], np.float32)
    W_q = np.asarray(inputs["W_q"], np.float32)
    W_k = np.asarray(inputs["W_k"], np.float32)
    W_v = np.asarray(inputs["W_v"], np.float32)
    W_o = np.asarray(inputs["W_o"], np.float32)
    b_q = np.asarray(inputs["b_q"], np.float32)

    bf = ml_dtypes.bfloat16

    def xblocks(x):  # [S, D] activations -> [blk, p, c, s] with x.T blocked
        xt = np.ascontiguousarray(x.T).astype(bf)  # [D, S]
        return np.ascontiguousarray(
            xt.reshape(CD, P, NQC, QC).transpose(2, 1, 0, 3)
        )

    def wblocks(w_sl):  # [D, 256] (= W[sl].T) -> [pr, p, c, d]
        return np.ascontiguousarray(
            w_sl.reshape(CD, P, 2, P).transpose(2, 1, 0, 3).astype(bf)
        )

    xq = [xblocks(q[b]) for b in range(B)]
    xk = [xblocks(k[b]) for b in range(B)]
    xv = [xblocks(v[b]) for b in range(B)]

    in_maps = []
    for core in range(NCORES):
        b, g = divmod(core, NG)
        sl = slice(g * DG, (g + 1) * DG)
        in_maps.append(
            {
                "xq": xq[b],
                "xk": xk[b],
                "xv": xv[b],
                "wq": wblocks(W_q[sl, :].T),
                "wk": wblocks(W_k[sl, :].T),
                "wv": wblocks(W_v[sl, :].T),
                "wo": np.ascontiguousarray(
                    W_o[:, sl].T.reshape(2, P, D).astype(bf)
                ),
                "bq": np.ascontiguousarray(b_q[sl].reshape(2, P).T),
                "ones2": np.ones((P, 2), bf),
            }
        )
    return in_maps


def kernel(**inputs):
    from concourse.bass_utils import run_bass_kernel_spmd

    nc = _get_program()
    in_maps = make_in_maps(inputs)
    trace = bool(int(os.environ.get("MHA_TRACE", "0")))
    res = run_bass_kernel_spmd(nc, in_maps, list(range(NCORES)), trace=trace)
    _CACHE["last_results"] = res

    W_o = np.asarray(inputs["W_o"], np.float64)
    b_o = np.asarray(inputs["b_o"], np.float64)
    b_v = np.asarray(inputs["b_v"], np.float64)
    out = np.zeros((B, S, D), np.float32)
    for core in range(NCORES):
        b = core // NG
        out[b] += res.results[core]["y"].astype(np.float32)
    # b_v and b_o commute with the attention average / output projection.
    out += (b_o + b_v @ W_o.T).astype(np.float32)[None, None, :]
    return out
